# revision 34
# baseline (speedup 1.0000x reference)
# Causal self-attention (B=2, S=2048, D=1024, H=16) on 8 TRN2 NeuronCores.
#
# Sharding: core = (batch b, head-group hg) with 4 heads per core — data
# parallel on B (cores 0-3 = batch 0, cores 4-7 = batch 1), tensor parallel
# on heads within each batch group. Per core:
#   1. Q/K projection (qk^T layout, tokens on free dim) for its 4 heads;
#      V projected directly in natural [token, dim] layout (lhsT = xT tile,
#      rhs = w_v) — no PE transposes; V bias via a K=1 ones-row matmul.
#   2. causal attention with the head PAIR interleaved in the k-loop:
#      the two heads of a pair live at partitions 0-63 / 64-127, so their
#      K=64 scores matmuls issue back-to-back and run concurrently in
#      different PE row groups; one Exp instruction covers both heads'
#      score banks ([128, 2, 512-t0] AP) halving ACT instruction count.
#      ctx^T accumulates with a ones-column appended to V so row 64 of the
#      ctx psum is the softmax denominator.
#   3. normalize: reciprocal_approx_fast on the denominator row (NOT the
#      56x slower iterative DVE reciprocal), broadcast via a K=1 matmul,
#      one DVE multiply into bf16 ctx_sb.
#   4. AllGather ctx^T over the 4-core batch group per 512-token chunk;
#      out-projection of chunk n is emitted AFTER attention of chunk n+1
#      so the gather wait never blocks queued PE work (engine FIFOs are
#      strict in-order). A dummy warmup AllGather absorbs the collective
#      cold-start during the input load phase.
# Host side shards/pre-transposes inputs and concatenates the 8 output
# column-slices; no host arithmetic beyond dtype casts and transposes.

import numpy as np
import ml_dtypes

import concourse.bass as bass
import concourse.mybir as mybir
import concourse.tile as tile
from concourse import bacc
from concourse.bass_utils import run_bass_kernel_spmd
from concourse.masks import make_upper_triangular

F32 = mybir.dt.float32
BF16 = mybir.dt.bfloat16

B, S, D, H, HD = 2, 2048, 1024, 16, 64
HG = 4                 # heads per core
DG = HG * HD           # 256 qkv cols per head-group
NCORES = 8
KT = 128               # key tile (partition dim of scoresT)
QC = 512               # query chunk (free dim of scoresT / psum width)
NKT = S // KT          # 16 key tiles
NQC = S // QC          # 4 query chunks
VW = HD + 2            # vnat stride per head (64 dims + ones col + pad,
                       # 66*2B keeps strided copy segments 4B-aligned)
SM_SCALE = 1.0 / 8.0   # 1/sqrt(HD)

# dtype knobs (matmul operand / storage dtypes; psums always fp32)
XW_DT = BF16           # x, w_qkv, and the Q^T/K^T tiles (scores matmul)
V_DT = BF16            # V natural tiles (ctx matmul lhsT)
ATTN_DT = BF16         # exp(scores) tiles (ctx matmul rhs)
CC_DT = BF16           # allgathered ctx^T
WOUT_DT = BF16         # out-projection weights

_NP = {BF16: ml_dtypes.bfloat16, F32: np.float32}

LAST_RESULTS = None    # BassKernelResults of the most recent kernel() call
_NC_CACHE = {}
DEBUG_OUTPUTS = False  # add per-stage debug outputs (dbg_qk/dbg_v/dbg_ctx/...)

KC = D // 128          # 8 contraction chunks for the projections


def _patch_act_tables():
    """Force Exp AND Ln onto the one set that has both
    (natural_log_exp_and_others) so the table-load pass emits a single
    load instead of thrashing between exp_and_others and natural_log
    (~2.7us per switch, 16 switches). Positions are preserved — the
    emitted act_func_set_id indexes the full act_info list."""
    import concourse.bacc as _bacc
    if getattr(_bacc, "_act_tables_patched", False):
        return
    orig = _bacc.get_activation_tables

    def patched(module_arch):
        tables = orig(module_arch)
        exp = mybir.ActivationFunctionType.Exp
        ln = mybir.ActivationFunctionType.Ln
        return {
            name: (set() if (name != "natural_log_exp_and_others"
                             and (exp in fns or ln in fns)) else fns)
            for name, fns in tables.items()
        }

    _bacc.get_activation_tables = patched
    _bacc._act_tables_patched = True


def _build_nc():
    _patch_act_tables()
    nc = bacc.Bacc(
        trn_type="TRN2",
        target_bir_lowering=False,
        debug=False,
        num_devices=NCORES,
    )

    xT = nc.declare_dram_parameter("xT", [D, S], XW_DT, isOutput=False)
    wqk = nc.declare_dram_parameter("wqk", [D, 2 * DG], XW_DT, isOutput=False)
    wv = nc.declare_dram_parameter("wv", [D, DG], XW_DT, isOutput=False)
    bqk = nc.declare_dram_parameter("bqk", [128, 4], F32, isOutput=False)
    bvrow = nc.declare_dram_parameter("bvrow", [1, DG], XW_DT, isOutput=False)
    wout = nc.declare_dram_parameter("wout", [D, DG], WOUT_DT, isOutput=False)
    bout = nc.declare_dram_parameter("bout", [128, 2], F32, isOutput=False)
    outT = nc.declare_dram_parameter("outT", [DG, S], F32, isOutput=True)
    if DEBUG_OUTPUTS:
        dbg_qk = nc.declare_dram_parameter(
            "dbg_qk", [128, 4 * S], BF16, isOutput=True)
        dbg_v = nc.declare_dram_parameter(
            "dbg_v", [128, NKT * HG * VW], BF16, isOutput=True)
        dbg_ctx = nc.declare_dram_parameter(
            "dbg_ctx", [128, 2 * S], BF16, isOutput=True)
        dbg_g = nc.declare_dram_parameter(
            "dbg_g", [128, 8 * S], BF16, isOutput=True)
        dbg_at = nc.declare_dram_parameter(
            "dbg_at", [128, 2 * QC], BF16, isOutput=True)
        dbg_rs = nc.declare_dram_parameter(
            "dbg_rs", [128, 16 * QC], F32, isOutput=True)

    with tile.TileContext(nc) as tc:
        with tc.tile_pool(name="persist", bufs=1) as ps:
            # ---- constants ----
            tri = ps.tile([128, 128], F32, tag="tri")
            make_upper_triangular(nc, tri, val=1.0, diag=True)
            tri_mm = ps.tile([128, 128], ATTN_DT, tag="tri_mm")
            nc.vector.tensor_copy(tri_mm, tri)
            ones1 = ps.tile([1, 128], ATTN_DT, tag="ones1")
            nc.vector.memset(ones1, 1.0)
            onesp = ps.tile([128, 64], ATTN_DT, tag="onesp")
            nc.vector.memset(onesp, 1.0)

            # ---- persistent SBUF tensors ----
            xT_sb = ps.tile([128, KC, S], XW_DT, tag="xT_sb")
            wqk_sb = ps.tile([128, KC, 2 * DG], XW_DT, tag="wqk_sb")
            wv_sb = ps.tile([128, KC, DG], XW_DT, tag="wv_sb")
            bqk_sb = ps.tile([128, 4], F32, tag="bqk_sb")
            bvrow_sb = ps.tile([1, DG], XW_DT, tag="bvrow_sb")
            qk_sb = ps.tile([128, 4, S], XW_DT, tag="qk_sb")      # Q^T,K^T
            vnat_sb = ps.tile([128, NKT, HG, VW], V_DT, tag="vnat_sb")
            ctx_sb = ps.tile([128, 2, S], CC_DT, tag="ctx_sb")    # normalized
            ctxg_sb = ps.tile([128, D // 128, S], CC_DT, tag="ctxg_sb")
            wout_sb = ps.tile([128, KC, DG], WOUT_DT, tag="wout_sb")
            bout_sb = ps.tile([128, 2], F32, tag="bout_sb")
            outT_sb = ps.tile([128, 2, S], F32, tag="outT_sb")

            nc.vector.memset(vnat_sb, 1.0)   # bakes the ones columns
            sums_pp = [ps.tile([33, QC], F32, tag=f"sums_pp{p}",
                               name=f"sums_pp{p}") for p in range(2)]
            nc.vector.memset(sums_pp[0], 1.0)
            nc.vector.memset(sums_pp[1], 1.0)

            if DEBUG_OUTPUTS:
                dbg_at_sb = ps.tile([128, 2, QC], ATTN_DT, tag="dbg_at_sb")
                dbg_rs_sb = ps.tile([128, 16, QC], F32, tag="dbg_rs_sb")
                nc.vector.memset(dbg_at_sb, 0.0)
                nc.vector.memset(dbg_rs_sb, 0.0)

            with tc.tile_pool(name="dram", bufs=1, space="DRAM") as dram:
                warm_in = dram.tile([128, 16], CC_DT, tag="warm_in",
                                    name="warm_in")
                warm_out = dram.tile([512, 16], CC_DT, tag="warm_out",
                                     name="warm_out")
                cc_in = [dram.tile([DG, QC], CC_DT, tag=f"cc_in{q}",
                                   name=f"cc_in{q}") for q in range(NQC)]
                cc_out = [dram.tile([D, QC], CC_DT, tag=f"cc_out{q}",
                                    name=f"cc_out{q}") for q in range(NQC)]

                # warmup collective: absorbs the ncfw cold-start + entry
                # barrier while the input DMAs stream in
                warm_sb = ps.tile([128, 16], CC_DT, tag="warm_sb")
                nc.vector.memset(warm_sb, 0.0)
                nc.sync.dma_start(out=warm_in[:], in_=warm_sb)
                nc.gpsimd.collective_compute(
                    "AllGather",
                    mybir.AluOpType.bypass,
                    replica_groups=[[0, 1, 2, 3], [4, 5, 6, 7]],
                    ins=[warm_in[:].opt()],
                    outs=[warm_out[:].opt()],
                )

                # ---- load inputs (chunk-0-critical pieces first) ----
                xT_r = xT.rearrange("(c p) s -> c p s", p=128)
                wqk_r = wqk.rearrange("(c p) m -> c p m", p=128)
                wv_r = wv.rearrange("(c p) m -> c p m", p=128)
                wout_r = wout.rearrange("(c p) m -> c p m", p=128)
                nc.sync.dma_start(out=bqk_sb, in_=bqk[:])
                nc.sync.dma_start(out=bvrow_sb, in_=bvrow[:])
                for c in range(KC):
                    nc.sync.dma_start(out=xT_sb[:, c, 0:QC],
                                      in_=xT_r[c][:, 0:QC])
                    nc.sync.dma_start(out=wqk_sb[:, c, :], in_=wqk_r[c])
                    nc.sync.dma_start(out=wv_sb[:, c, :], in_=wv_r[c])
                for c in range(KC):
                    nc.sync.dma_start(out=xT_sb[:, c, QC:S],
                                      in_=xT_r[c][:, QC:S])
                    nc.sync.dma_start(out=wout_sb[:, c, :], in_=wout_r[c])
                nc.sync.dma_start(out=bout_sb, in_=bout[:])

                def proj_chunk(n, gmm):
                    # Q/K for this chunk's tokens: m-chunk order q01 q23
                    # k01 k23 (heads 0,1 on partitions 0-63 / 64-127 of
                    # pair tiles)
                    for m in range(4):
                        pt = gmm.tile([128, QC], F32, tag="gemm")
                        for c in range(KC):
                            nc.tensor.matmul(
                                pt,
                                lhsT=wqk_sb[:, c, m * 128:(m + 1) * 128],
                                rhs=xT_sb[:, c, n * QC:(n + 1) * QC],
                                start=(c == 0),
                                stop=(c == KC - 1),
                            )
                        nc.vector.tensor_scalar_add(
                            qk_sb[:, m, n * QC:(n + 1) * QC], pt,
                            bqk_sb[:, m:m + 1])
                    # V natural directly: out[token, vdim], bias via K=1 mm
                    for t in range(4 * n, 4 * n + 4):
                        vt = gmm.tile([128, DG], F32, tag="gemm")
                        nc.tensor.matmul(
                            vt, lhsT=ones1[:, 0:128], rhs=bvrow_sb[:],
                            start=True, stop=False)
                        for c in range(KC):
                            nc.tensor.matmul(
                                vt,
                                lhsT=xT_sb[:, c, t * KT:(t + 1) * KT],
                                rhs=wv_sb[:, c, :],
                                start=False,
                                stop=(c == KC - 1),
                            )
                        nc.vector.tensor_copy(
                            vnat_sb[:, t, :, 0:HD],
                            vt.rearrange("p (h d) -> p h d", h=HG))

                def attention_pair(j, p, gmm, scp, cxp, asb, ssb):
                    # k-loop for head pair p (heads 2p, 2p+1) of chunk j,
                    # ending in the rs evictions + the pair's reciprocal.
                    # Returns (raws[2], recip2) for the deferred norm tail.
                    n_kt = 4 * j + 4      # key tiles 0 .. 4j+3
                    raws = []
                    if True:
                        sums2 = sums_pp[p]
                        cx = [cxp.tile([HD + 1, QC], F32, tag="cx",
                                       name=f"cx{z}")
                              for z in range(2)]
                        for i in range(n_kt):
                            tshift = KT * i - QC * j
                            t0 = max(tshift, 0)
                            sc = scp.tile([128, 2, QC], F32, tag="sc")
                            at = asb.tile([128, 2, QC], ATTN_DT, tag="attn")
                            for z in range(2):   # heads at po 0 / 64
                                po = 64 * z
                                nc.tensor.matmul(
                                    sc[:, z, t0:QC],
                                    lhsT=qk_sb[po:po + 64, 2 + p,
                                               i * KT:(i + 1) * KT],
                                    rhs=qk_sb[po:po + 64, p,
                                              j * QC + t0:(j + 1) * QC],
                                    start=True, stop=True,
                                )
                            nc.scalar.activation(
                                at[:, :, t0:QC], sc[:, :, t0:QC],
                                mybir.ActivationFunctionType.Exp,
                                scale=SM_SCALE,
                            )
                            if tshift >= 0:   # diagonal: mask k > q
                                for z in range(2):
                                    nc.vector.tensor_mul(
                                        at[:, z, t0:t0 + 128],
                                        at[:, z, t0:t0 + 128], tri_mm)
                            if DEBUG_OUTPUTS and j == 0 and p == 0 and i == 0:
                                nc.vector.tensor_copy(dbg_at_sb, at)
                            for z in range(2):
                                h = 2 * p + z
                                nc.tensor.matmul(
                                    cx[z][:, t0:QC],
                                    lhsT=vnat_sb[:, i, h, 0:HD + 1],
                                    rhs=at[:, z, t0:QC],
                                    start=(i == 0),
                                    stop=(i == n_kt - 1),
                                )
                        # evict raw ctx + sums in one copy per head (frees
                        # the psum banks for the next pair); sums rows are
                        # parked at partitions 32h of a shared tile for the
                        # chunk-batched reciprocal
                        for z in range(2):
                            h = 2 * p + z
                            rs = ssb.tile([HD + 1, QC], F32, tag="rs",
                                          bufs=8)
                            nc.vector.tensor_copy(rs, cx[z][:])
                            nc.vector.tensor_copy(
                                sums2[32 * z:32 * z + 1, :],
                                rs[HD:HD + 1, :])
                            if DEBUG_OUTPUTS:
                                hx = j * 4 + 2 * p + z
                                nc.vector.tensor_copy(
                                    dbg_rs_sb[0:HD + 1, hx, :], rs)
                            raws.append(rs)
                        # pair-batched reciprocal on ACT: 1/s = exp(-ln s);
                        # Ln and Exp share one activation table set
                        lnt = ssb.tile([33, QC], F32, tag="lnt", bufs=2)
                        nc.scalar.activation(
                            lnt, sums2[0:33, :],
                            mybir.ActivationFunctionType.Ln)
                        recip2 = ssb.tile([33, QC], ATTN_DT, tag="recip2",
                                          bufs=3)
                        with nc.allow_low_precision(
                                reason="softmax denominator broadcast"):
                            nc.scalar.activation(
                                recip2, lnt,
                                mybir.ActivationFunctionType.Exp,
                                scale=-1.0)
                    return raws, recip2

                def norm_tail(j, raws, recips, gmm):
                    # bc broadcast + final normalize multiply for chunk j.
                    # Emitted AFTER the next chunk's pair-A k-loop so these
                    # dependency-gated matmuls never head the PE FIFO.
                    for h in range(4):
                        p, z = h // 2, h % 2
                        if DEBUG_OUTPUTS:
                            nc.vector.tensor_copy(
                                dbg_rs_sb[96:97, j * 4 + h, :],
                                recips[p][32 * z:32 * z + 1, :])
                        bc = gmm.tile([64, QC], F32, tag="gemm")
                        nc.tensor.matmul(
                            bc, lhsT=onesp[32 * z:32 * z + 1, :],
                            rhs=recips[p][32 * z:32 * z + 1, :],
                            start=True, stop=True)
                        nc.vector.tensor_mul(
                            ctx_sb[64 * z:64 * z + 64, p,
                                   j * QC:(j + 1) * QC],
                            raws[h][0:HD, :], bc)

                def gather_chunk(q):
                    lo = q * QC
                    cc_in_r = cc_in[q].rearrange("(c p) s -> c p s", p=128)
                    for c in range(2):
                        nc.sync.dma_start(
                            out=cc_in_r[c], in_=ctx_sb[:, c, lo:lo + QC])
                    nc.gpsimd.collective_compute(
                        "AllGather",
                        mybir.AluOpType.bypass,
                        replica_groups=[[0, 1, 2, 3], [4, 5, 6, 7]],
                        ins=[cc_in[q][:].opt()],
                        outs=[cc_out[q][:].opt()],
                    )
                    cc_out_r = cc_out[q].rearrange("(c p) s -> c p s", p=128)
                    for c in range(D // 128):
                        nc.sync.dma_start(
                            out=ctxg_sb[:, c, lo:lo + QC], in_=cc_out_r[c])

                outT_r = outT.rearrange("(c p) s -> c p s", p=128)

                def out_proj_chunk(n, gmm):
                    for mo in range(2):
                        pt = gmm.tile([128, QC], F32, tag="gemm")
                        for c in range(KC):
                            nc.tensor.matmul(
                                pt,
                                lhsT=wout_sb[:, c, mo * 128:(mo + 1) * 128],
                                rhs=ctxg_sb[:, c, n * QC:(n + 1) * QC],
                                start=(c == 0),
                                stop=(c == KC - 1),
                            )
                        nc.vector.tensor_scalar_add(
                            outT_sb[:, mo, n * QC:(n + 1) * QC], pt,
                            bout_sb[:, mo:mo + 1])
                    for c in range(2):
                        nc.sync.dma_start(
                            out=outT_r[c][:, n * QC:(n + 1) * QC],
                            in_=outT_sb[:, c, n * QC:(n + 1) * QC])

                with tc.tile_pool(name="gemm_ps", bufs=2, space="PSUM") as gmm, \
                     tc.tile_pool(name="sc_ps", bufs=2, space="PSUM") as scp, \
                     tc.tile_pool(name="ctx_ps", bufs=2, space="PSUM") as cxp, \
                     tc.tile_pool(name="attn_sb", bufs=4) as asb, \
                     tc.tile_pool(name="small_sb", bufs=2) as ssb:
                    # software-pipelined emission: the norm tail of chunk n
                    # lands between pair-A and pair-B of chunk n+1, and
                    # out-proj n-1 after pair-B of chunk n+1, so every
                    # PE-FIFO head always has runnable work before it.
                    proj_chunk(0, gmm)
                    ra, recA = attention_pair(0, 0, gmm, scp, cxp, asb, ssb)
                    rb, recB = attention_pair(0, 1, gmm, scp, cxp, asb, ssb)
                    pending = (0, ra + rb, [recA, recB])
                    for n in range(NQC):
                        last = n == NQC - 1
                        if not last:
                            proj_chunk(n + 1, gmm)
                            ra, recA = attention_pair(
                                n + 1, 0, gmm, scp, cxp, asb, ssb)
                        norm_tail(pending[0], pending[1], pending[2], gmm)
                        gather_chunk(n)
                        if not last:
                            rb, recB = attention_pair(
                                n + 1, 1, gmm, scp, cxp, asb, ssb)
                            pending = (n + 1, ra + rb, [recA, recB])
                        if n > 0:
                            out_proj_chunk(n - 1, gmm)
                    out_proj_chunk(NQC - 1, gmm)

            if DEBUG_OUTPUTS:
                nc.sync.dma_start(
                    out=dbg_qk[:], in_=qk_sb.rearrange("p c s -> p (c s)"))
                nc.sync.dma_start(
                    out=dbg_v[:],
                    in_=vnat_sb.rearrange("p t h d -> p (t h d)"))
                nc.sync.dma_start(
                    out=dbg_ctx[:], in_=ctx_sb.rearrange("p c s -> p (c s)"))
                nc.sync.dma_start(
                    out=dbg_g[:], in_=ctxg_sb.rearrange("p c s -> p (c s)"))
                nc.sync.dma_start(
                    out=dbg_at[:],
                    in_=dbg_at_sb.rearrange("p c s -> p (c s)"))
                nc.sync.dma_start(
                    out=dbg_rs[:],
                    in_=dbg_rs_sb.rearrange("p c s -> p (c s)"))

    nc.compile()
    return nc


def get_nc():
    if "nc" not in _NC_CACHE:
        _NC_CACHE["nc"] = _build_nc()
    return _NC_CACHE["nc"]


def make_in_maps(x, w_qkv, b_qkv, w_out, b_out):
    x = np.asarray(x, np.float32)
    w_qkv = np.asarray(w_qkv, np.float32)
    b_qkv = np.asarray(b_qkv, np.float32)
    w_out = np.asarray(w_out, np.float32)
    b_out = np.asarray(b_out, np.float32)

    xw_np = _NP[XW_DT]
    wout_np = _NP[WOUT_DT]

    xT = [np.ascontiguousarray(x[b].T).astype(xw_np) for b in range(B)]
    in_maps = []
    for core in range(NCORES):
        b, hg = core // HG, core % HG
        sl = slice(hg * DG, (hg + 1) * DG)
        wq = w_qkv[:, sl]
        wk = w_qkv[:, D + hg * DG:D + (hg + 1) * DG]
        wv = w_qkv[:, 2 * D + hg * DG:2 * D + (hg + 1) * DG]
        bqk = np.concatenate(
            [b_qkv[sl], b_qkv[D + hg * DG:D + (hg + 1) * DG]])
        bv = b_qkv[2 * D + hg * DG:2 * D + (hg + 1) * DG]
        in_maps.append({
            "xT": xT[b],
            "wqk": np.ascontiguousarray(
                np.concatenate([wq, wk], axis=1)).astype(xw_np),
            "wv": np.ascontiguousarray(wv).astype(xw_np),
            "bqk": np.ascontiguousarray(
                bqk.reshape(4, 128).T).astype(np.float32),
            "bvrow": np.ascontiguousarray(bv.reshape(1, DG)).astype(xw_np),
            "wout": np.ascontiguousarray(w_out[:, sl]).astype(wout_np),
            "bout": np.ascontiguousarray(
                b_out[sl].reshape(2, 128).T).astype(np.float32),
        })
    return in_maps


def assemble_output(results):
    out = np.empty((B, S, D), np.float32)
    for core in range(NCORES):
        b, hg = core // HG, core % HG
        out[b, :, hg * DG:(hg + 1) * DG] = results[core]["outT"].T
    return out


def kernel(x, w_qkv, b_qkv, w_out, b_out):
    global LAST_RESULTS
    in_maps = make_in_maps(x, w_qkv, b_qkv, w_out, b_out)
    nc = get_nc()
    res = run_bass_kernel_spmd(nc, in_maps, list(range(NCORES)))
    LAST_RESULTS = res
    return assemble_output(res.results)


# revision 38
# speedup vs baseline: 1.0443x; 1.0443x over previous
# Causal self-attention (B=2, S=2048, D=1024, H=16) on 8 TRN2 NeuronCores.
#
# Sharding: core = (batch b, head-group hg) with 4 heads per core — data
# parallel on B (cores 0-3 = batch 0, cores 4-7 = batch 1), tensor parallel
# on heads within each batch group. Per core:
#   1. Q/K projection (qk^T layout, tokens on free dim) for its 4 heads;
#      V projected directly in natural [token, dim] layout (lhsT = xT tile,
#      rhs = w_v) — no PE transposes; V bias via a K=1 ones-row matmul.
#   2. causal attention with the head PAIR interleaved in the k-loop:
#      the two heads of a pair live at partitions 0-63 / 64-127, so their
#      K=64 scores matmuls issue back-to-back and run concurrently in
#      different PE row groups; one Exp instruction covers both heads'
#      score banks ([128, 2, 512-t0] AP) halving ACT instruction count.
#      ctx^T accumulates with a ones-column appended to V so row 64 of the
#      ctx psum is the softmax denominator.
#   3. normalize: reciprocal_approx_fast on the denominator row (NOT the
#      56x slower iterative DVE reciprocal), broadcast via a K=1 matmul,
#      one DVE multiply into bf16 ctx_sb.
#   4. AllGather ctx^T over the 4-core batch group per 512-token chunk;
#      out-projection of chunk n is emitted AFTER attention of chunk n+1
#      so the gather wait never blocks queued PE work (engine FIFOs are
#      strict in-order). A dummy warmup AllGather absorbs the collective
#      cold-start during the input load phase.
# Host side shards/pre-transposes inputs and concatenates the 8 output
# column-slices; no host arithmetic beyond dtype casts and transposes.

import numpy as np
import ml_dtypes

import concourse.bass as bass
import concourse.mybir as mybir
import concourse.tile as tile
from concourse import bacc
from concourse.bass_utils import run_bass_kernel_spmd
from concourse.masks import make_upper_triangular

F32 = mybir.dt.float32
BF16 = mybir.dt.bfloat16

B, S, D, H, HD = 2, 2048, 1024, 16, 64
HG = 4                 # heads per core
DG = HG * HD           # 256 qkv cols per head-group
NCORES = 8
KT = 128               # key tile (partition dim of scoresT)
QC = 512               # query chunk (free dim of scoresT / psum width)
NKT = S // KT          # 16 key tiles
NQC = S // QC          # 4 query chunks
VW = HD + 2            # vnat stride per head (64 dims + ones col + pad,
                       # 66*2B keeps strided copy segments 4B-aligned)
SM_SCALE = 1.0 / 8.0   # 1/sqrt(HD)

# dtype knobs (matmul operand / storage dtypes; psums always fp32)
XW_DT = BF16           # x, w_qkv, and the Q^T/K^T tiles (scores matmul)
V_DT = BF16            # V natural tiles (ctx matmul lhsT)
ATTN_DT = BF16         # exp(scores) tiles (ctx matmul rhs)
CC_DT = BF16           # allgathered ctx^T
WOUT_DT = BF16         # out-projection weights

_NP = {BF16: ml_dtypes.bfloat16, F32: np.float32}

LAST_RESULTS = None    # BassKernelResults of the most recent kernel() call
_NC_CACHE = {}
DEBUG_OUTPUTS = False  # add per-stage debug outputs (dbg_qk/dbg_v/dbg_ctx/...)

KC = D // 128          # 8 contraction chunks for the projections


def _patch_act_tables():
    """Force Exp AND Ln onto the one set that has both
    (natural_log_exp_and_others) so the table-load pass emits a single
    load instead of thrashing between exp_and_others and natural_log
    (~2.7us per switch, 16 switches). Positions are preserved — the
    emitted act_func_set_id indexes the full act_info list."""
    import concourse.bacc as _bacc
    if getattr(_bacc, "_act_tables_patched", False):
        return
    orig = _bacc.get_activation_tables

    def patched(module_arch):
        tables = orig(module_arch)
        exp = mybir.ActivationFunctionType.Exp
        ln = mybir.ActivationFunctionType.Ln
        return {
            name: (set() if (name != "natural_log_exp_and_others"
                             and (exp in fns or ln in fns)) else fns)
            for name, fns in tables.items()
        }

    _bacc.get_activation_tables = patched
    _bacc._act_tables_patched = True


def _build_nc():
    _patch_act_tables()
    nc = bacc.Bacc(
        trn_type="TRN2",
        target_bir_lowering=False,
        debug=False,
        num_devices=NCORES,
    )

    xT = nc.declare_dram_parameter("xT", [D, S], XW_DT, isOutput=False)
    wqk = nc.declare_dram_parameter("wqk", [D, 2 * DG], XW_DT, isOutput=False)
    wv = nc.declare_dram_parameter("wv", [D, DG], XW_DT, isOutput=False)
    bqk = nc.declare_dram_parameter("bqk", [128, 4], F32, isOutput=False)
    bvrow = nc.declare_dram_parameter("bvrow", [1, DG], XW_DT, isOutput=False)
    wout = nc.declare_dram_parameter("wout", [D, DG], WOUT_DT, isOutput=False)
    bout = nc.declare_dram_parameter("bout", [128, 2], F32, isOutput=False)
    outT = nc.declare_dram_parameter("outT", [DG, S], F32, isOutput=True)
    if DEBUG_OUTPUTS:
        dbg_qk = nc.declare_dram_parameter(
            "dbg_qk", [128, 4 * S], BF16, isOutput=True)
        dbg_v = nc.declare_dram_parameter(
            "dbg_v", [128, NKT * HG * VW], BF16, isOutput=True)
        dbg_ctx = nc.declare_dram_parameter(
            "dbg_ctx", [128, 2 * S], BF16, isOutput=True)
        dbg_g = nc.declare_dram_parameter(
            "dbg_g", [128, 8 * S], BF16, isOutput=True)
        dbg_at = nc.declare_dram_parameter(
            "dbg_at", [128, 2 * QC], BF16, isOutput=True)
        dbg_rs = nc.declare_dram_parameter(
            "dbg_rs", [128, 16 * QC], F32, isOutput=True)

    with tile.TileContext(nc) as tc:
        with tc.tile_pool(name="persist", bufs=1) as ps:
            # ---- constants ----
            tri = ps.tile([128, 128], F32, tag="tri")
            make_upper_triangular(nc, tri, val=1.0, diag=True)
            tri_mm = ps.tile([128, 128], ATTN_DT, tag="tri_mm")
            nc.vector.tensor_copy(tri_mm, tri)
            ones1 = ps.tile([1, 128], ATTN_DT, tag="ones1")
            nc.vector.memset(ones1, 1.0)
            onesp = ps.tile([128, 64], ATTN_DT, tag="onesp")
            nc.vector.memset(onesp, 1.0)

            # ---- persistent SBUF tensors ----
            xT_sb = ps.tile([128, KC, S], XW_DT, tag="xT_sb")
            wqk_sb = ps.tile([128, KC, 2 * DG], XW_DT, tag="wqk_sb")
            wv_sb = ps.tile([128, KC, DG], XW_DT, tag="wv_sb")
            bqk_sb = ps.tile([128, 4], F32, tag="bqk_sb")
            bvrow_sb = ps.tile([1, DG], XW_DT, tag="bvrow_sb")
            qk_sb = ps.tile([128, 4, S], XW_DT, tag="qk_sb")      # Q^T,K^T
            vnat_sb = ps.tile([128, NKT, HG, VW], V_DT, tag="vnat_sb")
            ctx_sb = ps.tile([128, 2, S], CC_DT, tag="ctx_sb")    # normalized
            ctxg_sb = ps.tile([128, D // 128, S], CC_DT, tag="ctxg_sb")
            wout_sb = ps.tile([128, KC, DG], WOUT_DT, tag="wout_sb")
            bout_sb = ps.tile([128, 2], F32, tag="bout_sb")
            outT_sb = ps.tile([128, 2, S], F32, tag="outT_sb")

            nc.vector.memset(vnat_sb, 1.0)   # bakes the ones columns
            sums_pp = [ps.tile([33, QC], F32, tag=f"sums_pp{p}",
                               name=f"sums_pp{p}") for p in range(2)]
            nc.vector.memset(sums_pp[0], 1.0)
            nc.vector.memset(sums_pp[1], 1.0)

            if DEBUG_OUTPUTS:
                dbg_at_sb = ps.tile([128, 2, QC], ATTN_DT, tag="dbg_at_sb")
                dbg_rs_sb = ps.tile([128, 16, QC], F32, tag="dbg_rs_sb")
                nc.vector.memset(dbg_at_sb, 0.0)
                nc.vector.memset(dbg_rs_sb, 0.0)

            with tc.tile_pool(name="dram", bufs=1, space="DRAM") as dram:
                cc_in = [dram.tile([DG, QC], CC_DT, tag=f"cc_in{q}",
                                   name=f"cc_in{q}") for q in range(NQC)]
                cc_out = [dram.tile([D, QC], CC_DT, tag=f"cc_out{q}",
                                    name=f"cc_out{q}") for q in range(NQC)]

                # ---- load inputs (chunk-0-critical pieces first) ----
                xT_r = xT.rearrange("(c p) s -> c p s", p=128)
                wqk_r = wqk.rearrange("(c p) m -> c p m", p=128)
                wv_r = wv.rearrange("(c p) m -> c p m", p=128)
                wout_r = wout.rearrange("(c p) m -> c p m", p=128)
                nc.sync.dma_start(out=bqk_sb, in_=bqk[:])
                nc.sync.dma_start(out=bvrow_sb, in_=bvrow[:])
                for c in range(KC):
                    nc.sync.dma_start(out=xT_sb[:, c, 0:QC],
                                      in_=xT_r[c][:, 0:QC])
                    nc.sync.dma_start(out=wqk_sb[:, c, :], in_=wqk_r[c])
                    nc.sync.dma_start(out=wv_sb[:, c, :], in_=wv_r[c])
                for c in range(KC):
                    nc.sync.dma_start(out=xT_sb[:, c, QC:S],
                                      in_=xT_r[c][:, QC:S])
                    nc.sync.dma_start(out=wout_sb[:, c, :], in_=wout_r[c])
                nc.sync.dma_start(out=bout_sb, in_=bout[:])

                def proj_chunk(n, gmm):
                    # Q/K for this chunk's tokens: m-chunk order q01 q23
                    # k01 k23 (heads 0,1 on partitions 0-63 / 64-127 of
                    # pair tiles)
                    for m in range(4):
                        pt = gmm.tile([128, QC], F32, tag="gemm")
                        for c in range(KC):
                            nc.tensor.matmul(
                                pt,
                                lhsT=wqk_sb[:, c, m * 128:(m + 1) * 128],
                                rhs=xT_sb[:, c, n * QC:(n + 1) * QC],
                                start=(c == 0),
                                stop=(c == KC - 1),
                            )
                        nc.vector.tensor_scalar_add(
                            qk_sb[:, m, n * QC:(n + 1) * QC], pt,
                            bqk_sb[:, m:m + 1])
                    # V natural directly: out[token, vdim], bias via K=1 mm
                    for t in range(4 * n, 4 * n + 4):
                        vt = gmm.tile([128, DG], F32, tag="gemm")
                        nc.tensor.matmul(
                            vt, lhsT=ones1[:, 0:128], rhs=bvrow_sb[:],
                            start=True, stop=False)
                        for c in range(KC):
                            nc.tensor.matmul(
                                vt,
                                lhsT=xT_sb[:, c, t * KT:(t + 1) * KT],
                                rhs=wv_sb[:, c, :],
                                start=False,
                                stop=(c == KC - 1),
                            )
                        nc.vector.tensor_copy(
                            vnat_sb[:, t, :, 0:HD],
                            vt.rearrange("p (h d) -> p h d", h=HG))

                def attention_pair(j, p, gmm, scp, cxp, asb, ssb):
                    # k-loop for head pair p (heads 2p, 2p+1) of chunk j,
                    # ending in the rs evictions + the pair's reciprocal.
                    # Returns (raws[2], recip2) for the deferred norm tail.
                    n_kt = 4 * j + 4      # key tiles 0 .. 4j+3
                    raws = []
                    if True:
                        sums2 = sums_pp[p]
                        cx = [cxp.tile([HD + 1, QC], F32, tag="cx",
                                       name=f"cx{z}")
                              for z in range(2)]
                        for i in range(n_kt):
                            tshift = KT * i - QC * j
                            t0 = max(tshift, 0)
                            sc = scp.tile([128, 2, QC], F32, tag="sc")
                            at = asb.tile([128, 2, QC], ATTN_DT, tag="attn")
                            for z in range(2):   # heads at po 0 / 64
                                po = 64 * z
                                nc.tensor.matmul(
                                    sc[:, z, t0:QC],
                                    lhsT=qk_sb[po:po + 64, 2 + p,
                                               i * KT:(i + 1) * KT],
                                    rhs=qk_sb[po:po + 64, p,
                                              j * QC + t0:(j + 1) * QC],
                                    start=True, stop=True,
                                )
                            nc.scalar.activation(
                                at[:, :, t0:QC], sc[:, :, t0:QC],
                                mybir.ActivationFunctionType.Exp,
                                scale=SM_SCALE,
                            )
                            if tshift >= 0:   # diagonal: mask k > q
                                for z in range(2):
                                    nc.vector.tensor_mul(
                                        at[:, z, t0:t0 + 128],
                                        at[:, z, t0:t0 + 128], tri_mm)
                            if DEBUG_OUTPUTS and j == 0 and p == 0 and i == 0:
                                nc.vector.tensor_copy(dbg_at_sb, at)
                            for z in range(2):
                                h = 2 * p + z
                                nc.tensor.matmul(
                                    cx[z][:, t0:QC],
                                    lhsT=vnat_sb[:, i, h, 0:HD + 1],
                                    rhs=at[:, z, t0:QC],
                                    start=(i == 0),
                                    stop=(i == n_kt - 1),
                                )
                        # evict raw ctx + sums in one copy per head (frees
                        # the psum banks for the next pair); sums rows are
                        # parked at partitions 32h of a shared tile for the
                        # chunk-batched reciprocal
                        for z in range(2):
                            h = 2 * p + z
                            rs = ssb.tile([HD + 1, QC], F32, tag="rs",
                                          bufs=8)
                            nc.vector.tensor_copy(rs, cx[z][:])
                            nc.vector.tensor_copy(
                                sums2[32 * z:32 * z + 1, :],
                                cx[z][HD:HD + 1, :])
                            if DEBUG_OUTPUTS:
                                hx = j * 4 + 2 * p + z
                                nc.vector.tensor_copy(
                                    dbg_rs_sb[0:HD + 1, hx, :], rs)
                            raws.append(rs)
                        # pair-batched reciprocal on ACT: 1/s = exp(-ln s);
                        # Ln and Exp share one activation table set
                        lnt = ssb.tile([33, QC], F32, tag="lnt", bufs=2)
                        nc.scalar.activation(
                            lnt, sums2[0:33, :],
                            mybir.ActivationFunctionType.Ln)
                        recip2 = ssb.tile([33, QC], ATTN_DT, tag="recip2",
                                          bufs=3)
                        with nc.allow_low_precision(
                                reason="softmax denominator broadcast"):
                            nc.scalar.activation(
                                recip2, lnt,
                                mybir.ActivationFunctionType.Exp,
                                scale=-1.0)
                    return raws, recip2

                def norm_tail(j, raws, recips, gmm):
                    # bc broadcast + final normalize multiply for chunk j.
                    # Emitted AFTER the next chunk's pair-A k-loop so these
                    # dependency-gated matmuls never head the PE FIFO.
                    for h in range(4):
                        p, z = h // 2, h % 2
                        if DEBUG_OUTPUTS:
                            nc.vector.tensor_copy(
                                dbg_rs_sb[96:97, j * 4 + h, :],
                                recips[p][32 * z:32 * z + 1, :])
                        bc = gmm.tile([64, QC], F32, tag="gemm")
                        nc.tensor.matmul(
                            bc, lhsT=onesp[32 * z:32 * z + 1, :],
                            rhs=recips[p][32 * z:32 * z + 1, :],
                            start=True, stop=True)
                        nc.vector.tensor_mul(
                            ctx_sb[64 * z:64 * z + 64, p,
                                   j * QC:(j + 1) * QC],
                            raws[h][0:HD, :], bc)

                def gather_chunk(q):
                    lo = q * QC
                    cc_in_r = cc_in[q].rearrange("(c p) s -> c p s", p=128)
                    for c in range(2):
                        nc.sync.dma_start(
                            out=cc_in_r[c], in_=ctx_sb[:, c, lo:lo + QC])
                    nc.gpsimd.collective_compute(
                        "AllGather",
                        mybir.AluOpType.bypass,
                        replica_groups=[[0, 1, 2, 3], [4, 5, 6, 7]],
                        ins=[cc_in[q][:].opt()],
                        outs=[cc_out[q][:].opt()],
                    )
                    # inbound on the (idle) GpSimd queue: their wait on AG
                    # completion must not head-block the Sync DMA FIFO
                    cc_out_r = cc_out[q].rearrange("(c p) s -> c p s", p=128)
                    for c in range(D // 128):
                        nc.gpsimd.dma_start(
                            out=ctxg_sb[:, c, lo:lo + QC], in_=cc_out_r[c])

                outT_r = outT.rearrange("(c p) s -> c p s", p=128)

                def out_proj_chunk(n, gmm):
                    for mo in range(2):
                        pt = gmm.tile([128, QC], F32, tag="gemm")
                        for c in range(KC):
                            nc.tensor.matmul(
                                pt,
                                lhsT=wout_sb[:, c, mo * 128:(mo + 1) * 128],
                                rhs=ctxg_sb[:, c, n * QC:(n + 1) * QC],
                                start=(c == 0),
                                stop=(c == KC - 1),
                            )
                        nc.vector.tensor_scalar_add(
                            outT_sb[:, mo, n * QC:(n + 1) * QC], pt,
                            bout_sb[:, mo:mo + 1])
                    for c in range(2):
                        nc.sync.dma_start(
                            out=outT_r[c][:, n * QC:(n + 1) * QC],
                            in_=outT_sb[:, c, n * QC:(n + 1) * QC])

                with tc.tile_pool(name="gemm_ps", bufs=2, space="PSUM") as gmm, \
                     tc.tile_pool(name="sc_ps", bufs=2, space="PSUM") as scp, \
                     tc.tile_pool(name="ctx_ps", bufs=2, space="PSUM") as cxp, \
                     tc.tile_pool(name="attn_sb", bufs=4) as asb, \
                     tc.tile_pool(name="small_sb", bufs=2) as ssb:
                    # software-pipelined emission: the norm tail of chunk n
                    # lands between pair-A and pair-B of chunk n+1, and
                    # out-proj n-1 after pair-B of chunk n+1, so every
                    # PE-FIFO head always has runnable work before it.
                    proj_chunk(0, gmm)
                    ra, recA = attention_pair(0, 0, gmm, scp, cxp, asb, ssb)
                    rb, recB = attention_pair(0, 1, gmm, scp, cxp, asb, ssb)
                    pending = (0, ra + rb, [recA, recB])
                    for n in range(NQC):
                        last = n == NQC - 1
                        if not last:
                            proj_chunk(n + 1, gmm)
                            ra, recA = attention_pair(
                                n + 1, 0, gmm, scp, cxp, asb, ssb)
                        norm_tail(pending[0], pending[1], pending[2], gmm)
                        gather_chunk(n)
                        if not last:
                            rb, recB = attention_pair(
                                n + 1, 1, gmm, scp, cxp, asb, ssb)
                            pending = (n + 1, ra + rb, [recA, recB])
                        if n > 0:
                            out_proj_chunk(n - 1, gmm)
                    out_proj_chunk(NQC - 1, gmm)

            if DEBUG_OUTPUTS:
                nc.sync.dma_start(
                    out=dbg_qk[:], in_=qk_sb.rearrange("p c s -> p (c s)"))
                nc.sync.dma_start(
                    out=dbg_v[:],
                    in_=vnat_sb.rearrange("p t h d -> p (t h d)"))
                nc.sync.dma_start(
                    out=dbg_ctx[:], in_=ctx_sb.rearrange("p c s -> p (c s)"))
                nc.sync.dma_start(
                    out=dbg_g[:], in_=ctxg_sb.rearrange("p c s -> p (c s)"))
                nc.sync.dma_start(
                    out=dbg_at[:],
                    in_=dbg_at_sb.rearrange("p c s -> p (c s)"))
                nc.sync.dma_start(
                    out=dbg_rs[:],
                    in_=dbg_rs_sb.rearrange("p c s -> p (c s)"))

    nc.compile()
    return nc


def get_nc():
    if "nc" not in _NC_CACHE:
        _NC_CACHE["nc"] = _build_nc()
    return _NC_CACHE["nc"]


def make_in_maps(x, w_qkv, b_qkv, w_out, b_out):
    x = np.asarray(x, np.float32)
    w_qkv = np.asarray(w_qkv, np.float32)
    b_qkv = np.asarray(b_qkv, np.float32)
    w_out = np.asarray(w_out, np.float32)
    b_out = np.asarray(b_out, np.float32)

    xw_np = _NP[XW_DT]
    wout_np = _NP[WOUT_DT]

    xT = [np.ascontiguousarray(x[b].T).astype(xw_np) for b in range(B)]
    in_maps = []
    for core in range(NCORES):
        b, hg = core // HG, core % HG
        sl = slice(hg * DG, (hg + 1) * DG)
        wq = w_qkv[:, sl]
        wk = w_qkv[:, D + hg * DG:D + (hg + 1) * DG]
        wv = w_qkv[:, 2 * D + hg * DG:2 * D + (hg + 1) * DG]
        bqk = np.concatenate(
            [b_qkv[sl], b_qkv[D + hg * DG:D + (hg + 1) * DG]])
        bv = b_qkv[2 * D + hg * DG:2 * D + (hg + 1) * DG]
        in_maps.append({
            "xT": xT[b],
            "wqk": np.ascontiguousarray(
                np.concatenate([wq, wk], axis=1)).astype(xw_np),
            "wv": np.ascontiguousarray(wv).astype(xw_np),
            "bqk": np.ascontiguousarray(
                bqk.reshape(4, 128).T).astype(np.float32),
            "bvrow": np.ascontiguousarray(bv.reshape(1, DG)).astype(xw_np),
            "wout": np.ascontiguousarray(w_out[:, sl]).astype(wout_np),
            "bout": np.ascontiguousarray(
                b_out[sl].reshape(2, 128).T).astype(np.float32),
        })
    return in_maps


def assemble_output(results):
    out = np.empty((B, S, D), np.float32)
    for core in range(NCORES):
        b, hg = core // HG, core % HG
        out[b, :, hg * DG:(hg + 1) * DG] = results[core]["outT"].T
    return out


def kernel(x, w_qkv, b_qkv, w_out, b_out):
    global LAST_RESULTS
    in_maps = make_in_maps(x, w_qkv, b_qkv, w_out, b_out)
    nc = get_nc()
    res = run_bass_kernel_spmd(nc, in_maps, list(range(NCORES)))
    LAST_RESULTS = res
    return assemble_output(res.results)


# revision 39
# speedup vs baseline: 1.0734x; 1.0279x over previous
# Causal self-attention (B=2, S=2048, D=1024, H=16) on 8 TRN2 NeuronCores.
#
# Sharding: core = (batch b, head-group hg) with 4 heads per core — data
# parallel on B (cores 0-3 = batch 0, cores 4-7 = batch 1), tensor parallel
# on heads within each batch group. Per core:
#   1. Q/K projection (qk^T layout, tokens on free dim) for its 4 heads;
#      V projected directly in natural [token, dim] layout (lhsT = xT tile,
#      rhs = w_v) — no PE transposes; V bias via a K=1 ones-row matmul.
#   2. causal attention with the head PAIR interleaved in the k-loop:
#      the two heads of a pair live at partitions 0-63 / 64-127, so their
#      K=64 scores matmuls issue back-to-back and run concurrently in
#      different PE row groups; one Exp instruction covers both heads'
#      score banks ([128, 2, 512-t0] AP) halving ACT instruction count.
#      ctx^T accumulates with a ones-column appended to V so row 64 of the
#      ctx psum is the softmax denominator.
#   3. normalize: reciprocal_approx_fast on the denominator row (NOT the
#      56x slower iterative DVE reciprocal), broadcast via a K=1 matmul,
#      one DVE multiply into bf16 ctx_sb.
#   4. AllGather ctx^T over the 4-core batch group per 512-token chunk;
#      out-projection of chunk n is emitted AFTER attention of chunk n+1
#      so the gather wait never blocks queued PE work (engine FIFOs are
#      strict in-order). A dummy warmup AllGather absorbs the collective
#      cold-start during the input load phase.
# Host side shards/pre-transposes inputs and concatenates the 8 output
# column-slices; no host arithmetic beyond dtype casts and transposes.

import numpy as np
import ml_dtypes

import concourse.bass as bass
import concourse.mybir as mybir
import concourse.tile as tile
from concourse import bacc
from concourse.bass_utils import run_bass_kernel_spmd
from concourse.masks import make_upper_triangular

F32 = mybir.dt.float32
BF16 = mybir.dt.bfloat16

B, S, D, H, HD = 2, 2048, 1024, 16, 64
HG = 4                 # heads per core
DG = HG * HD           # 256 qkv cols per head-group
NCORES = 8
KT = 128               # key tile (partition dim of scoresT)
QC = 512               # query chunk (free dim of scoresT / psum width)
NKT = S // KT          # 16 key tiles
NQC = S // QC          # 4 query chunks
VW = HD + 2            # vnat stride per head (64 dims + ones col + pad,
                       # 66*2B keeps strided copy segments 4B-aligned)
SM_SCALE = 1.0 / 8.0   # 1/sqrt(HD)

# dtype knobs (matmul operand / storage dtypes; psums always fp32)
XW_DT = BF16           # x, w_qkv, and the Q^T/K^T tiles (scores matmul)
V_DT = BF16            # V natural tiles (ctx matmul lhsT)
ATTN_DT = BF16         # exp(scores) tiles (ctx matmul rhs)
CC_DT = BF16           # allgathered ctx^T
WOUT_DT = BF16         # out-projection weights

_NP = {BF16: ml_dtypes.bfloat16, F32: np.float32}

LAST_RESULTS = None    # BassKernelResults of the most recent kernel() call
_NC_CACHE = {}
DEBUG_OUTPUTS = False  # add per-stage debug outputs (dbg_qk/dbg_v/dbg_ctx/...)

KC = D // 128          # 8 contraction chunks for the projections


def _patch_act_tables():
    """Force Exp AND Ln onto the one set that has both
    (natural_log_exp_and_others) so the table-load pass emits a single
    load instead of thrashing between exp_and_others and natural_log
    (~2.7us per switch, 16 switches). Positions are preserved — the
    emitted act_func_set_id indexes the full act_info list."""
    import concourse.bacc as _bacc
    if getattr(_bacc, "_act_tables_patched", False):
        return
    orig = _bacc.get_activation_tables

    def patched(module_arch):
        tables = orig(module_arch)
        exp = mybir.ActivationFunctionType.Exp
        ln = mybir.ActivationFunctionType.Ln
        return {
            name: (set() if (name != "natural_log_exp_and_others"
                             and (exp in fns or ln in fns)) else fns)
            for name, fns in tables.items()
        }

    _bacc.get_activation_tables = patched
    _bacc._act_tables_patched = True


def _build_nc():
    _patch_act_tables()
    nc = bacc.Bacc(
        trn_type="TRN2",
        target_bir_lowering=False,
        debug=False,
        num_devices=NCORES,
    )

    xT = nc.declare_dram_parameter("xT", [D, S], XW_DT, isOutput=False)
    wqk = nc.declare_dram_parameter("wqk", [D, 2 * DG], XW_DT, isOutput=False)
    wv = nc.declare_dram_parameter("wv", [D, DG], XW_DT, isOutput=False)
    bqk = nc.declare_dram_parameter("bqk", [128, 4], F32, isOutput=False)
    bvrow = nc.declare_dram_parameter("bvrow", [1, DG], XW_DT, isOutput=False)
    wout = nc.declare_dram_parameter("wout", [D, DG], WOUT_DT, isOutput=False)
    bout = nc.declare_dram_parameter("bout", [128, 2], F32, isOutput=False)
    outT = nc.declare_dram_parameter("outT", [DG, S], F32, isOutput=True)
    if DEBUG_OUTPUTS:
        dbg_qk = nc.declare_dram_parameter(
            "dbg_qk", [128, 4 * S], BF16, isOutput=True)
        dbg_v = nc.declare_dram_parameter(
            "dbg_v", [128, NKT * HG * VW], BF16, isOutput=True)
        dbg_ctx = nc.declare_dram_parameter(
            "dbg_ctx", [128, 2 * S], BF16, isOutput=True)
        dbg_g = nc.declare_dram_parameter(
            "dbg_g", [128, 8 * S], BF16, isOutput=True)
        dbg_at = nc.declare_dram_parameter(
            "dbg_at", [128, 2 * QC], BF16, isOutput=True)
        dbg_rs = nc.declare_dram_parameter(
            "dbg_rs", [128, 16 * QC], F32, isOutput=True)

    with tile.TileContext(nc) as tc:
        with tc.tile_pool(name="persist", bufs=1) as ps:
            # ---- constants ----
            tri = ps.tile([128, 128], F32, tag="tri")
            make_upper_triangular(nc, tri, val=1.0, diag=True)
            tri_mm = ps.tile([128, 128], ATTN_DT, tag="tri_mm")
            nc.vector.tensor_copy(tri_mm, tri)
            ones1 = ps.tile([1, 128], ATTN_DT, tag="ones1")
            nc.vector.memset(ones1, 1.0)
            onesp = ps.tile([128, 64], ATTN_DT, tag="onesp")
            nc.vector.memset(onesp, 1.0)

            # ---- persistent SBUF tensors ----
            xT_sb = ps.tile([128, KC, S], XW_DT, tag="xT_sb")
            wqk_sb = ps.tile([128, KC, 2 * DG], XW_DT, tag="wqk_sb")
            wv_sb = ps.tile([128, KC, DG], XW_DT, tag="wv_sb")
            bqk_sb = ps.tile([128, 4], F32, tag="bqk_sb")
            bvrow_sb = ps.tile([1, DG], XW_DT, tag="bvrow_sb")
            qk_sb = ps.tile([128, 4, S], XW_DT, tag="qk_sb")      # Q^T,K^T
            vnat_sb = ps.tile([128, NKT, HG, VW], V_DT, tag="vnat_sb")
            ctx_sb = ps.tile([128, 2, S], CC_DT, tag="ctx_sb")    # normalized
            ctxg_sb = ps.tile([128, D // 128, S], CC_DT, tag="ctxg_sb")
            wout_sb = ps.tile([128, KC, DG], WOUT_DT, tag="wout_sb")
            bout_sb = ps.tile([128, 2], F32, tag="bout_sb")
            outT_sb = ps.tile([128, 2, S], F32, tag="outT_sb")

            nc.vector.memset(vnat_sb, 1.0)   # bakes the ones columns
            sums_pp = [ps.tile([33, QC], F32, tag=f"sums_pp{p}",
                               name=f"sums_pp{p}") for p in range(2)]
            nc.vector.memset(sums_pp[0], 1.0)
            nc.vector.memset(sums_pp[1], 1.0)

            if DEBUG_OUTPUTS:
                dbg_at_sb = ps.tile([128, 2, QC], ATTN_DT, tag="dbg_at_sb")
                dbg_rs_sb = ps.tile([128, 16, QC], F32, tag="dbg_rs_sb")
                nc.vector.memset(dbg_at_sb, 0.0)
                nc.vector.memset(dbg_rs_sb, 0.0)

            with tc.tile_pool(name="dram", bufs=1, space="DRAM") as dram:
                warm_in = dram.tile([128, 16], CC_DT, tag="warm_in",
                                    name="warm_in")
                warm_out = dram.tile([512, 16], CC_DT, tag="warm_out",
                                     name="warm_out")
                cc_in = [dram.tile([DG, QC], CC_DT, tag=f"cc_in{q}",
                                   name=f"cc_in{q}") for q in range(NQC)]
                cc_out = [dram.tile([D, QC], CC_DT, tag=f"cc_out{q}",
                                    name=f"cc_out{q}") for q in range(NQC)]

                # warmup collective: absorbs the ncfw cold-start + entry
                # barrier while the input DMAs stream in
                warm_sb = ps.tile([128, 16], CC_DT, tag="warm_sb")
                nc.vector.memset(warm_sb, 0.0)
                nc.sync.dma_start(out=warm_in[:], in_=warm_sb)
                nc.gpsimd.collective_compute(
                    "AllGather",
                    mybir.AluOpType.bypass,
                    replica_groups=[[0, 1, 2, 3], [4, 5, 6, 7]],
                    ins=[warm_in[:].opt()],
                    outs=[warm_out[:].opt()],
                )

                # ---- load inputs (chunk-0-critical pieces first) ----
                xT_r = xT.rearrange("(c p) s -> c p s", p=128)
                wqk_r = wqk.rearrange("(c p) m -> c p m", p=128)
                wv_r = wv.rearrange("(c p) m -> c p m", p=128)
                wout_r = wout.rearrange("(c p) m -> c p m", p=128)
                nc.sync.dma_start(out=bqk_sb, in_=bqk[:])
                nc.sync.dma_start(out=bvrow_sb, in_=bvrow[:])
                for c in range(KC):
                    nc.sync.dma_start(out=xT_sb[:, c, 0:QC],
                                      in_=xT_r[c][:, 0:QC])
                    nc.sync.dma_start(out=wqk_sb[:, c, :], in_=wqk_r[c])
                    nc.sync.dma_start(out=wv_sb[:, c, :], in_=wv_r[c])
                for c in range(KC):
                    nc.sync.dma_start(out=xT_sb[:, c, QC:S],
                                      in_=xT_r[c][:, QC:S])
                    nc.sync.dma_start(out=wout_sb[:, c, :], in_=wout_r[c])
                nc.sync.dma_start(out=bout_sb, in_=bout[:])

                def proj_chunk(n, gmm):
                    # Q/K for this chunk's tokens: m-chunk order q01 q23
                    # k01 k23 (heads 0,1 on partitions 0-63 / 64-127 of
                    # pair tiles)
                    for m in range(4):
                        pt = gmm.tile([128, QC], F32, tag="gemm")
                        for c in range(KC):
                            nc.tensor.matmul(
                                pt,
                                lhsT=wqk_sb[:, c, m * 128:(m + 1) * 128],
                                rhs=xT_sb[:, c, n * QC:(n + 1) * QC],
                                start=(c == 0),
                                stop=(c == KC - 1),
                            )
                        nc.vector.tensor_scalar_add(
                            qk_sb[:, m, n * QC:(n + 1) * QC], pt,
                            bqk_sb[:, m:m + 1])
                    # V natural directly: out[token, vdim], bias via K=1 mm
                    for t in range(4 * n, 4 * n + 4):
                        vt = gmm.tile([128, DG], F32, tag="gemm")
                        nc.tensor.matmul(
                            vt, lhsT=ones1[:, 0:128], rhs=bvrow_sb[:],
                            start=True, stop=False)
                        for c in range(KC):
                            nc.tensor.matmul(
                                vt,
                                lhsT=xT_sb[:, c, t * KT:(t + 1) * KT],
                                rhs=wv_sb[:, c, :],
                                start=False,
                                stop=(c == KC - 1),
                            )
                        nc.vector.tensor_copy(
                            vnat_sb[:, t, :, 0:HD],
                            vt.rearrange("p (h d) -> p h d", h=HG))

                def attention_pair(j, p, gmm, scp, cxp, asb, ssb):
                    # k-loop for head pair p (heads 2p, 2p+1) of chunk j,
                    # ending in the rs evictions + the pair's reciprocal.
                    # Returns (raws[2], recip2) for the deferred norm tail.
                    n_kt = 4 * j + 4      # key tiles 0 .. 4j+3
                    raws = []
                    if True:
                        sums2 = sums_pp[p]
                        cx = [cxp.tile([HD + 1, QC], F32, tag="cx",
                                       name=f"cx{z}")
                              for z in range(2)]
                        for i in range(n_kt):
                            tshift = KT * i - QC * j
                            t0 = max(tshift, 0)
                            sc = scp.tile([128, 2, QC], F32, tag="sc")
                            at = asb.tile([128, 2, QC], ATTN_DT, tag="attn")
                            for z in range(2):   # heads at po 0 / 64
                                po = 64 * z
                                nc.tensor.matmul(
                                    sc[:, z, t0:QC],
                                    lhsT=qk_sb[po:po + 64, 2 + p,
                                               i * KT:(i + 1) * KT],
                                    rhs=qk_sb[po:po + 64, p,
                                              j * QC + t0:(j + 1) * QC],
                                    start=True, stop=True,
                                )
                            nc.scalar.activation(
                                at[:, :, t0:QC], sc[:, :, t0:QC],
                                mybir.ActivationFunctionType.Exp,
                                scale=SM_SCALE,
                            )
                            if tshift >= 0:   # diagonal: mask k > q
                                for z in range(2):
                                    nc.vector.tensor_mul(
                                        at[:, z, t0:t0 + 128],
                                        at[:, z, t0:t0 + 128], tri_mm)
                            if DEBUG_OUTPUTS and j == 0 and p == 0 and i == 0:
                                nc.vector.tensor_copy(dbg_at_sb, at)
                            for z in range(2):
                                h = 2 * p + z
                                nc.tensor.matmul(
                                    cx[z][:, t0:QC],
                                    lhsT=vnat_sb[:, i, h, 0:HD + 1],
                                    rhs=at[:, z, t0:QC],
                                    start=(i == 0),
                                    stop=(i == n_kt - 1),
                                )
                        # evict raw ctx + sums in one copy per head (frees
                        # the psum banks for the next pair); sums rows are
                        # parked at partitions 32h of a shared tile for the
                        # chunk-batched reciprocal
                        for z in range(2):
                            h = 2 * p + z
                            rs = ssb.tile([HD + 1, QC], F32, tag="rs",
                                          bufs=8)
                            nc.vector.tensor_copy(rs, cx[z][:])
                            nc.vector.tensor_copy(
                                sums2[32 * z:32 * z + 1, :],
                                cx[z][HD:HD + 1, :])
                            if DEBUG_OUTPUTS:
                                hx = j * 4 + 2 * p + z
                                nc.vector.tensor_copy(
                                    dbg_rs_sb[0:HD + 1, hx, :], rs)
                            raws.append(rs)
                        # pair-batched reciprocal on ACT: 1/s = exp(-ln s);
                        # Ln and Exp share one activation table set
                        lnt = ssb.tile([33, QC], F32, tag="lnt", bufs=2)
                        nc.scalar.activation(
                            lnt, sums2[0:33, :],
                            mybir.ActivationFunctionType.Ln)
                        recip2 = ssb.tile([33, QC], ATTN_DT, tag="recip2",
                                          bufs=3)
                        with nc.allow_low_precision(
                                reason="softmax denominator broadcast"):
                            nc.scalar.activation(
                                recip2, lnt,
                                mybir.ActivationFunctionType.Exp,
                                scale=-1.0)
                    return raws, recip2

                def norm_tail(j, raws, recips, gmm):
                    # bc broadcast + final normalize multiply for chunk j.
                    # Emitted AFTER the next chunk's pair-A k-loop so these
                    # dependency-gated matmuls never head the PE FIFO.
                    for h in range(4):
                        p, z = h // 2, h % 2
                        if DEBUG_OUTPUTS:
                            nc.vector.tensor_copy(
                                dbg_rs_sb[96:97, j * 4 + h, :],
                                recips[p][32 * z:32 * z + 1, :])
                        bc = gmm.tile([64, QC], F32, tag="gemm")
                        nc.tensor.matmul(
                            bc, lhsT=onesp[32 * z:32 * z + 1, :],
                            rhs=recips[p][32 * z:32 * z + 1, :],
                            start=True, stop=True)
                        nc.vector.tensor_mul(
                            ctx_sb[64 * z:64 * z + 64, p,
                                   j * QC:(j + 1) * QC],
                            raws[h][0:HD, :], bc)

                def gather_chunk(q):
                    lo = q * QC
                    cc_in_r = cc_in[q].rearrange("(c p) s -> c p s", p=128)
                    for c in range(2):
                        nc.sync.dma_start(
                            out=cc_in_r[c], in_=ctx_sb[:, c, lo:lo + QC])
                    nc.gpsimd.collective_compute(
                        "AllGather",
                        mybir.AluOpType.bypass,
                        replica_groups=[[0, 1, 2, 3], [4, 5, 6, 7]],
                        ins=[cc_in[q][:].opt()],
                        outs=[cc_out[q][:].opt()],
                    )
                    # inbound on the (idle) GpSimd queue: their wait on AG
                    # completion must not head-block the Sync DMA FIFO
                    cc_out_r = cc_out[q].rearrange("(c p) s -> c p s", p=128)
                    for c in range(D // 128):
                        nc.gpsimd.dma_start(
                            out=ctxg_sb[:, c, lo:lo + QC], in_=cc_out_r[c])

                outT_r = outT.rearrange("(c p) s -> c p s", p=128)

                def out_proj_chunk(n, gmm):
                    for mo in range(2):
                        pt = gmm.tile([128, QC], F32, tag="gemm")
                        for c in range(KC):
                            nc.tensor.matmul(
                                pt,
                                lhsT=wout_sb[:, c, mo * 128:(mo + 1) * 128],
                                rhs=ctxg_sb[:, c, n * QC:(n + 1) * QC],
                                start=(c == 0),
                                stop=(c == KC - 1),
                            )
                        nc.vector.tensor_scalar_add(
                            outT_sb[:, mo, n * QC:(n + 1) * QC], pt,
                            bout_sb[:, mo:mo + 1])
                    for c in range(2):
                        nc.sync.dma_start(
                            out=outT_r[c][:, n * QC:(n + 1) * QC],
                            in_=outT_sb[:, c, n * QC:(n + 1) * QC])

                with tc.tile_pool(name="gemm_ps", bufs=2, space="PSUM") as gmm, \
                     tc.tile_pool(name="sc_ps", bufs=2, space="PSUM") as scp, \
                     tc.tile_pool(name="ctx_ps", bufs=2, space="PSUM") as cxp, \
                     tc.tile_pool(name="attn_sb", bufs=4) as asb, \
                     tc.tile_pool(name="small_sb", bufs=2) as ssb:
                    # software-pipelined emission: the norm tail of chunk n
                    # lands between pair-A and pair-B of chunk n+1, and
                    # out-proj n-1 after pair-B of chunk n+1, so every
                    # PE-FIFO head always has runnable work before it.
                    proj_chunk(0, gmm)
                    ra, recA = attention_pair(0, 0, gmm, scp, cxp, asb, ssb)
                    rb, recB = attention_pair(0, 1, gmm, scp, cxp, asb, ssb)
                    pending = (0, ra + rb, [recA, recB])
                    for n in range(NQC):
                        last = n == NQC - 1
                        if not last:
                            proj_chunk(n + 1, gmm)
                            ra, recA = attention_pair(
                                n + 1, 0, gmm, scp, cxp, asb, ssb)
                        norm_tail(pending[0], pending[1], pending[2], gmm)
                        gather_chunk(n)
                        if not last:
                            rb, recB = attention_pair(
                                n + 1, 1, gmm, scp, cxp, asb, ssb)
                            pending = (n + 1, ra + rb, [recA, recB])
                        if n > 0:
                            out_proj_chunk(n - 1, gmm)
                    out_proj_chunk(NQC - 1, gmm)

            if DEBUG_OUTPUTS:
                nc.sync.dma_start(
                    out=dbg_qk[:], in_=qk_sb.rearrange("p c s -> p (c s)"))
                nc.sync.dma_start(
                    out=dbg_v[:],
                    in_=vnat_sb.rearrange("p t h d -> p (t h d)"))
                nc.sync.dma_start(
                    out=dbg_ctx[:], in_=ctx_sb.rearrange("p c s -> p (c s)"))
                nc.sync.dma_start(
                    out=dbg_g[:], in_=ctxg_sb.rearrange("p c s -> p (c s)"))
                nc.sync.dma_start(
                    out=dbg_at[:],
                    in_=dbg_at_sb.rearrange("p c s -> p (c s)"))
                nc.sync.dma_start(
                    out=dbg_rs[:],
                    in_=dbg_rs_sb.rearrange("p c s -> p (c s)"))

    nc.compile()
    return nc


def get_nc():
    if "nc" not in _NC_CACHE:
        _NC_CACHE["nc"] = _build_nc()
    return _NC_CACHE["nc"]


def make_in_maps(x, w_qkv, b_qkv, w_out, b_out):
    x = np.asarray(x, np.float32)
    w_qkv = np.asarray(w_qkv, np.float32)
    b_qkv = np.asarray(b_qkv, np.float32)
    w_out = np.asarray(w_out, np.float32)
    b_out = np.asarray(b_out, np.float32)

    xw_np = _NP[XW_DT]
    wout_np = _NP[WOUT_DT]

    xT = [np.ascontiguousarray(x[b].T).astype(xw_np) for b in range(B)]
    in_maps = []
    for core in range(NCORES):
        b, hg = core // HG, core % HG
        sl = slice(hg * DG, (hg + 1) * DG)
        wq = w_qkv[:, sl]
        wk = w_qkv[:, D + hg * DG:D + (hg + 1) * DG]
        wv = w_qkv[:, 2 * D + hg * DG:2 * D + (hg + 1) * DG]
        bqk = np.concatenate(
            [b_qkv[sl], b_qkv[D + hg * DG:D + (hg + 1) * DG]])
        bv = b_qkv[2 * D + hg * DG:2 * D + (hg + 1) * DG]
        in_maps.append({
            "xT": xT[b],
            "wqk": np.ascontiguousarray(
                np.concatenate([wq, wk], axis=1)).astype(xw_np),
            "wv": np.ascontiguousarray(wv).astype(xw_np),
            "bqk": np.ascontiguousarray(
                bqk.reshape(4, 128).T).astype(np.float32),
            "bvrow": np.ascontiguousarray(bv.reshape(1, DG)).astype(xw_np),
            "wout": np.ascontiguousarray(w_out[:, sl]).astype(wout_np),
            "bout": np.ascontiguousarray(
                b_out[sl].reshape(2, 128).T).astype(np.float32),
        })
    return in_maps


def assemble_output(results):
    out = np.empty((B, S, D), np.float32)
    for core in range(NCORES):
        b, hg = core // HG, core % HG
        out[b, :, hg * DG:(hg + 1) * DG] = results[core]["outT"].T
    return out


def kernel(x, w_qkv, b_qkv, w_out, b_out):
    global LAST_RESULTS
    in_maps = make_in_maps(x, w_qkv, b_qkv, w_out, b_out)
    nc = get_nc()
    res = run_bass_kernel_spmd(nc, in_maps, list(range(NCORES)))
    LAST_RESULTS = res
    return assemble_output(res.results)
